# revision 24
# baseline (speedup 1.0000x reference)
"""Causal self-attention (B=4, T=2048, C=768, H=12) on 8 Trainium2 cores.

Sharding: core c handles batch b=c//2 and heads [6*(c%2), 6*(c%2)+6).
Each core computes its 6 heads end-to-end (qkv proj -> attention -> partial
c_proj); the host sums the two partial c_proj outputs per batch and adds the
bias (v-bias is folded into the host-side bias since softmax weights sum to 1).

All matmul operands are fp16 (1 cycle/row on PE at any width, half the DMA
of f32); accumulation stays fp32 in PSUM.

Attention per head:
  S.T[k,q] = K.T @ Q per 128-key block (keys on partitions);
  es = exp(S.T * 1/sqrt(D)) fused on ScalarE, fp16 out;
  causal mask via affine_select on the diagonal 128x128 block only;
  PV transposed: O[q,d] = es(kb-block).T @ V[kb] accumulated over kb in PSUM,
  65-wide rhs per head (64 v-dims + ones column -> softmax denominators land
  per-partition);
  normalization = strided reciprocal + per-head tensor_scalar_mul (denom is a
  per-partition scalar in this layout - no broadcast machinery needed);
  O[q,d] pairs of heads are flipped to [d,q] for c_proj via DMA-engine
  transposes (idle resource; no PE/DVE cost).

Emission order is produced by a greedy dual-clock scheduler that paces S
pieces to keep ScalarE's exp stream (~19us/head) saturated while filling PE
with projection / PV / c_proj work. Head 0 runs descending kb (so S can start
as soon as the tail token chunks arrive from HBM); later heads run ascending
kb so PV(qb) unlocks progressively after exp(kb<=qb).
"""

import sys

sys.path.insert(0, "/opt/trn_rl_repo")

from contextlib import ExitStack

import numpy as np

import concourse.bass as bass
import concourse.tile as tile
from concourse import bacc, mybir, bass_utils

B, T, C, H = 4, 2048, 768, 12
D = C // H  # 64
HPC = H // 2  # heads per core = 6
NCORES = 8
CB = C // 128  # 6 contraction chunks
KB = T // 128  # 16 key blocks
TB = T // 128  # 16 token blocks
VC = HPC * (D + 1)  # 390 v cols incl ones
SPAN = HPC * D  # 384

f32 = mybir.dt.float32
f16 = mybir.dt.float16
ts = bass.ts
SCALE = 1.0 / float(np.sqrt(D))
Exp = mybir.ActivationFunctionType.Exp

PE_NS = 0.4167  # ns per matmul row at full pstate
ACT_NS = 0.8333


def _emit(tc, xT, wqk, bqk, wv, wpc, y, dbg=None):
    nc = tc.nc

    with ExitStack() as top:
        xw = top.enter_context(tc.tile_pool(name="xw", bufs=1))
        wp = top.enter_context(tc.tile_pool(name="wp", bufs=1))
        qkp = top.enter_context(tc.tile_pool(name="qkp", bufs=1))
        vtp = top.enter_context(tc.tile_pool(name="vtp", bufs=1))
        esp = top.enter_context(tc.tile_pool(name="esp", bufs=2))
        osb = top.enter_context(tc.tile_pool(name="osb", bufs=2))
        rp = top.enter_context(tc.tile_pool(name="rp", bufs=4))
        ocp = top.enter_context(tc.tile_pool(name="ocp", bufs=1))
        yop = top.enter_context(tc.tile_pool(name="yop", bufs=3))

        stp = top.enter_context(tc.tile_pool(name="stp", bufs=3, space="PSUM"))
        opp = top.enter_context(tc.tile_pool(name="opp", bufs=2, space="PSUM"))

        xt = [xw.tile([128, T], f16, tag=f"xt{i}", name=f"xt{i}") for i in range(CB)]
        wqkt = [wp.tile([128, C], f16, tag=f"wqk{i}", name=f"wqk{i}") for i in range(CB)]
        bqa = wp.tile([128, CB], f32, tag="bqa", name="bqa")
        wvt = [wp.tile([128, SPAN], f16, tag=f"wv{i}", name=f"wv{i}") for i in range(CB)]
        qkt = [qkp.tile([128, T], f16, tag=f"qkt{i}", name=f"qkt{i}") for i in range(CB)]
        vt = [vtp.tile([128, VC], f16, tag=f"vt{t}", name=f"vt{t}") for t in range(TB)]
        ocat = [ocp.tile([128, T], f16, tag=f"oc{i}", name=f"oc{i}") for i in range(3)]
        wpt = [wp.tile([128, C], f16, tag=f"wp{i}", name=f"wp{i}") for i in range(3)]

        # ---------------- DMA loads (order = serial DMA-device order) -------
        dma_t = 0.0

        def dma_cost(nbytes):
            return nbytes / 360.0  # 16 engines x 22.5 B/ns

        xt_ready = {}  # tch -> est ready ns
        for i in range(CB):
            nc.sync.dma_start(wqkt[i][:], wqk[ts(i, 128), :])
            dma_t += max(dma_cost(128 * C * 2), 625)
        # single gathered bias DMA: bqa[p, i] = bqk[128*i + p]
        nc.sync.dma_start(bqa[:], bqk.rearrange("(i p) o -> p (i o)", i=CB, p=128))
        dma_t += 625
        wqk_ready = dma_t
        for i in range(CB):
            nc.sync.dma_start(xt[i][:, 0:1024], xT[ts(i, 128), 0:1024])
            dma_t += max(dma_cost(128 * 1024 * 2), 625)
        xt_ready[0] = xt_ready[1] = dma_t
        for i in range(CB):
            nc.sync.dma_start(xt[i][:, 1024:2048], xT[ts(i, 128), 1024:2048])
            dma_t += max(dma_cost(128 * 1024 * 2), 625)
        xt_ready[2] = xt_ready[3] = dma_t
        for i in range(CB):
            nc.sync.dma_start(wvt[i][:], wv[ts(i, 128), :])
            dma_t += 625
        wv_ready = dma_t
        for i in range(3):
            nc.sync.dma_start(wpt[i][:], wpc[ts(i, 128), :])

        # ---------------- emitters ----------------
        def emit_qk_proj(ob, tch):
            ps = stp.tile([128, 512], f32, tag="st", name="pj")
            for kc in range(CB):
                nc.tensor.matmul(
                    ps[:],
                    wqkt[kc][:, ts(ob, 128)],
                    xt[kc][:, ts(tch, 512)],
                    start=(kc == 0),
                    stop=(kc == CB - 1),
                )
            nc.vector.tensor_scalar_add(
                qkt[ob][:, ts(tch, 512)], ps[:], bqa[:, ob : ob + 1]
            )

        def emit_v_proj(tb):
            ps = stp.tile([128, SPAN], f32, tag="st", name="pjv")
            for kc in range(CB):
                nc.tensor.matmul(
                    ps[:],
                    xt[kc][:, ts(tb, 128)],
                    wvt[kc][:],
                    start=(kc == 0),
                    stop=(kc == CB - 1),
                )
            nc.vector.tensor_copy(
                vt[tb][:].rearrange("p (h d) -> p h d", h=HPC, d=65)[:, :, 0:D],
                ps[:].rearrange("p (h d) -> p h d", h=HPC, d=D),
            )
            nc.gpsimd.memset(vt[tb][:, D:VC:65], 1.0)

        es_tiles = {}  # (h, kb) -> tile

        def emit_s_piece(h, kb, p0, w, last):
            bp = D * (h % 2)
            qt = qkt[h // 2]
            kt = qkt[3 + h // 2][bp : bp + D, ts(kb, 128)]
            base = 128 * kb
            if (h, kb) not in es_tiles:
                es_tiles[(h, kb)] = esp.tile(
                    [128, T - base], f16, tag=f"es{kb}", name=f"es{kb}_{h}"
                )
            e = es_tiles[(h, kb)]
            st = stp.tile([128, w], f32, tag="st", name="st")
            for sub in range(p0, p0 + w, 512):
                sw = min(512, p0 + w - sub)
                nc.tensor.matmul(
                    st[:, sub - p0 : sub - p0 + sw],
                    kt,
                    qt[bp : bp + D, sub : sub + sw],
                    start=True,
                    stop=True,
                )
            nc.scalar.activation(
                e[:, p0 - base : p0 - base + w], st[:], Exp, scale=SCALE
            )
            if p0 == base:
                # causal mask on the diagonal block: keep q >= k
                nc.gpsimd.affine_select(
                    out=e[:, 0:128],
                    in_=e[:, 0:128],
                    compare_op=mybir.AluOpType.is_ge,
                    fill=0.0,
                    base=0,
                    pattern=[[1, 128]],
                    channel_multiplier=-1,
                )

        op_cur = [None]

        def emit_pv(h, qb):
            j = qb % 4
            if j == 0:
                op_cur[0] = opp.tile([128, 260], f32, tag="op", name="op")
            op = op_cur[0]
            c0 = 65 * j
            for kb in range(qb + 1):
                nc.tensor.matmul(
                    op[:, c0 : c0 + 65],
                    es_tiles[(h, kb)][:, 128 * (qb - kb) : 128 * (qb - kb) + 128],
                    vt[kb][:, 65 * h : 65 * h + 65],
                    start=(j == 0 and kb == 0),
                    stop=(kb == qb),
                )

        def emit_norm(h, g, o2s):
            # op tile for group g is the one allocated at pv(h,4g); norms are
            # emitted right after pv(h,4g+3), so op_cur is correct.
            op = op_cur[0]
            r = rp.tile([128, 4], f32, tag="r", name="r")
            nc.vector.reciprocal(r[:], op[:, D : 260 : 65])
            bp = D * (h % 2)
            for j in range(4):
                qb = 4 * g + j
                nc.vector.tensor_scalar_mul(
                    o2s[qb][:, bp : bp + D], op[:, 65 * j : 65 * j + D], r[:, j : j + 1]
                )
                if h % 2 == 1:
                    nc.sync.dma_start_transpose(
                        ocat[h // 2][:, ts(qb, 128)], o2s[qb][:]
                    )

        def emit_cproj(tb):
            ps = stp.tile([128, C], f32, tag="st", name="yps")
            for kc in range(3):
                for a, wdt in ((0, 512), (512, 256)):
                    nc.tensor.matmul(
                        ps[:, a : a + wdt],
                        ocat[kc][:, ts(tb, 128)],
                        wpt[kc][:, a : a + wdt],
                        start=(kc == 0),
                        stop=(kc == 2),
                    )
            yt = yop.tile([128, C], f32, tag="yt", name="yt")
            nc.vector.tensor_copy(yt[:], ps[:])
            nc.sync.dma_start(y[ts(tb, 128), :], yt[:])

        # ---------------- greedy dual-clock schedule ----------------
        # units
        projs = [(0, 0), (3, 0), (0, 1), (3, 1), (0, 2), (3, 2), (0, 3), (3, 3),
                 (1, 0), (4, 0), (1, 1), (4, 1), (1, 2), (4, 2), (1, 3), (4, 3),
                 (2, 0), (5, 0), (2, 1), (5, 1), (2, 2), (5, 2), (2, 3), (5, 3)]
        vprojs = list(range(TB))  # ascending: pv(h,qb) needs vt[0..qb]

        def pieces_for(h):
            # pieces split at the absolute 1024 boundary; all lo-half pieces
            # first (they only need the first xt DMA wave), then hi halves
            lo, hi = [], []
            for kb in range(KB):
                base = 128 * kb
                if base < 1024:
                    lo.append((kb, base, 1024 - base, False))
                    hi.append((kb, 1024, 1024, True))
                else:
                    hi.append((kb, base, T - base, True))
            return lo + hi

        spieces = {h: pieces_for(h) for h in range(HPC)}
        sp_ptr = {h: 0 for h in range(HPC)}
        covered = {}        # (h, kb) -> max q col emitted
        pv_ptr = {h: 0 for h in range(HPC)}
        norm_ptr = {h: 0 for h in range(HPC)}
        phase2_done = {h: False for h in range(HPC)}
        o2tiles = {}
        cprojs = list(range(TB))
        cp_ptr = [0]
        pi, vi = [0], [0]

        pe_t, act_t = [0.0], [0.0]

        emitted_projs = set()

        def proj_ready(i):
            ob, tch = projs[i]
            return max(wqk_ready, xt_ready[tch])

        def vproj_ready(i):
            tb = vprojs[i]
            return max(wv_ready, xt_ready[tb // 4])

        def s_gate_ok(h):
            if sp_ptr[h] >= len(spieces[h]):
                return False
            if h >= 2 and not phase2_done[h - 2]:
                return False
            kb, p0, w, _ = spieces[h][sp_ptr[h]]
            need = {(h // 2, t) for t in range(p0 // 512, (p0 + w + 511) // 512)}
            need.add((3 + h // 2, kb // 4))
            return need <= emitted_projs

        def do_s(h):
            kb, p0, w, last = spieces[h][sp_ptr[h]]
            emit_s_piece(h, kb, p0, w, last)
            sp_ptr[h] += 1
            pe_t[0] += w * PE_NS
            act_t[0] = max(act_t[0], pe_t[0] + 100) + w * ACT_NS + 242
            covered[(h, kb)] = p0 + w

        def pv_gate_ok(h):
            qb = pv_ptr[h]
            if qb >= TB or vi[0] <= qb:
                return False
            need = 128 * (qb + 1)
            return all(covered.get((h, kb), 0) >= need for kb in range(qb + 1))

        def do_pv(h):
            qb = pv_ptr[h]
            if h % 2 == 0 and qb == 0:
                p = h // 2
                o2tiles[p] = [
                    osb.tile([128, 128], f16, tag=f"o2q{q}", name=f"o2q{q}_{p}")
                    for q in range(TB)
                ]
            emit_pv(h, qb)
            pv_ptr[h] += 1
            pe_t[0] += (qb + 1) * 65 * PE_NS
            if qb % 4 == 3:
                emit_norm(h, qb // 4, o2tiles[h // 2])
                norm_ptr[h] += 1
                if qb == TB - 1:
                    phase2_done[h] = True

        def emit_all():
            while True:
                # eager: cproj after head-5 norms
                while cp_ptr[0] < TB and norm_ptr[HPC - 1] > cprojs[cp_ptr[0]] // 4:
                    emit_cproj(cprojs[cp_ptr[0]])
                    cp_ptr[0] += 1
                    pe_t[0] += 2304 * PE_NS

                # pick active head for s; pv is strictly head-major sequential
                # (all pv chains share the rotating op PSUM tile)
                s_h = next((h for h in range(HPC) if s_gate_ok(h)), None)
                cur = next((h for h in range(HPC) if pv_ptr[h] < TB), None)
                pv_h = cur if cur is not None and pv_gate_ok(cur) else None

                def do_proj():
                    ob, tch = projs[pi[0]]
                    emit_qk_proj(ob, tch)
                    emitted_projs.add((ob, tch))
                    pi[0] += 1
                    pe_t[0] = max(pe_t[0], proj_ready(pi[0] - 1)) + 512 * CB * PE_NS

                def do_vproj():
                    emit_v_proj(vprojs[vi[0]])
                    vi[0] += 1
                    pe_t[0] = max(pe_t[0], vproj_ready(vi[0] - 1)) + SPAN * CB * PE_NS

                proj_ok = pi[0] < len(projs) and pe_t[0] >= proj_ready(pi[0]) - 500
                vproj_ok = vi[0] < len(vprojs) and pe_t[0] >= vproj_ready(vi[0]) - 500

                if s_h is not None and act_t[0] - pe_t[0] < 1200:
                    do_s(s_h)
                elif pv_h is not None:
                    do_pv(pv_h)
                elif vproj_ok:
                    do_vproj()
                elif proj_ok:
                    do_proj()
                elif s_h is not None:
                    do_s(s_h)
                elif pi[0] < len(projs) or vi[0] < len(vprojs):
                    # all remaining work DMA-gated: emit whichever is ready
                    # soonest (PE will wait on its semaphore)
                    pr = proj_ready(pi[0]) if pi[0] < len(projs) else 1e18
                    vr = vproj_ready(vi[0]) if vi[0] < len(vprojs) else 1e18
                    if pr <= vr:
                        do_proj()
                    else:
                        do_vproj()
                elif cp_ptr[0] < TB:
                    # tail cprojs not yet unlocked: should not happen (norms
                    # are emitted inside do_pv); emit remaining directly
                    emit_cproj(cprojs[cp_ptr[0]])
                    cp_ptr[0] += 1
                else:
                    break

        emit_all()

        if dbg is not None:
            for i in range(CB):
                nc.sync.dma_start(dbg["qkT"][ts(i, 128), :], qkt[i][:])
            for t in range(TB):
                nc.sync.dma_start(dbg["v"][ts(t, 128), :], vt[t][:])
            for i in range(3):
                nc.sync.dma_start(dbg["oc"][ts(i, 128), :], ocat[i][:])


_PROGRAM = None


def _build(dbg=False):
    global _PROGRAM
    if _PROGRAM is not None and not dbg:
        return _PROGRAM
    nc = bacc.Bacc("TRN2", target_bir_lowering=False, debug=False, num_devices=NCORES)
    xT = nc.dram_tensor("xT", [C, T], f16, kind="ExternalInput").ap()
    wqk = nc.dram_tensor("wqk", [C, C], f16, kind="ExternalInput").ap()
    bqk = nc.dram_tensor("bqk", [C, 1], f32, kind="ExternalInput").ap()
    wv = nc.dram_tensor("wv", [C, SPAN], f16, kind="ExternalInput").ap()
    wpc = nc.dram_tensor("wpc", [SPAN, C], f16, kind="ExternalInput").ap()
    y = nc.dram_tensor("y", [T, C], f32, kind="ExternalOutput").ap()
    dbgd = None
    if dbg:
        dbgd = {
            "qkT": nc.dram_tensor("dbg_qkT", [C, T], f16, kind="ExternalOutput").ap(),
            "v": nc.dram_tensor("dbg_v", [T, VC], f16, kind="ExternalOutput").ap(),
            "oc": nc.dram_tensor("dbg_oc", [SPAN, T], f16, kind="ExternalOutput").ap(),
        }
    with tile.TileContext(nc) as tc:
        _emit(tc, xT, wqk, bqk, wv, wpc, y, dbg=dbgd)
    nc.compile()
    if not dbg:
        _PROGRAM = nc
    return nc


def _in_maps(x, w_qkv, b_qkv, w_proj):
    maps = []
    for c in range(NCORES):
        b = c // 2
        half = c % 2
        r0 = SPAN * half

        wq = w_qkv[r0 : r0 + SPAN, :]
        wk = w_qkv[C + r0 : C + r0 + SPAN, :]
        wqk = np.ascontiguousarray(np.vstack([wq, wk]).T)  # [C, 768]
        bqk = np.concatenate(
            [b_qkv[r0 : r0 + SPAN], b_qkv[C + r0 : C + r0 + SPAN]]
        ).reshape(C, 1)
        wv = np.ascontiguousarray(w_qkv[2 * C + r0 : 2 * C + r0 + SPAN, :].T)
        wpc = np.ascontiguousarray(w_proj[:, r0 : r0 + SPAN].T)  # [384, C]

        maps.append(
            {
                "xT": np.ascontiguousarray(x[b].T).astype(np.float16),
                "wqk": wqk.astype(np.float16),
                "bqk": bqk.astype(np.float32),
                "wv": wv.astype(np.float16),
                "wpc": wpc.astype(np.float16),
            }
        )
    return maps


def kernel(x, w_qkv, b_qkv, w_proj, b_proj, _trace=False):
    x = np.asarray(x, dtype=np.float32)
    w_qkv = np.asarray(w_qkv, dtype=np.float32)
    b_qkv = np.asarray(b_qkv, dtype=np.float32)
    w_proj = np.asarray(w_proj, dtype=np.float32)
    b_proj = np.asarray(b_proj, dtype=np.float32)

    nc = _build()
    maps = _in_maps(x, w_qkv, b_qkv, w_proj)
    res = bass_utils.run_bass_kernel_spmd(
        nc, maps, core_ids=list(range(NCORES)), trace=_trace
    )
    # v-bias contributes sum_k a_k * bv = bv per token; fold through c_proj.
    b_eff = b_proj + w_proj @ b_qkv[2 * C : 3 * C]
    out = np.empty((B, T, C), dtype=np.float32)
    for b in range(B):
        out[b] = res.results[2 * b]["y"] + res.results[2 * b + 1]["y"] + b_eff
    if _trace:
        kernel._last_exec_time_ns = res.exec_time_ns
        kernel._last_results = res
    return out
